# revision 1
# baseline (speedup 1.0000x reference)
"""Peephole-LSTM Trainium2 kernel (Bass/Tile), batch-parallel over 8 cores.

Problem: B=32, T=2048, F=128, H=256.
  xw = x @ Wx.T + b  (precomputed on device, bf16, SBUF-resident)
  per step: gates = xw_t + h @ Wh.T (+ peepholes), fp32 state, bf16 matmuls.

Layouts (per core, BC=4 batch rows):
  xT DRAM (128, T*4)      x^T, col = t*4+b, bf16
  xw SBUF (128, T*32)     col = s*32 + m*4 + b, m = gate tile (i,i,f,f,o,o,g,g)
  state tiles (128, 8)    col = half*4 + b   (half = gate-dim half)
  out_h/out_c DRAM (128, T*8)  col = s*8 + half*4 + b, fp32
"""

import numpy as np
import ml_dtypes

import concourse.bass as bass
import concourse.bacc as bacc
import concourse.mybir as mybir
import concourse.tile as tile
from concourse.bass_utils import run_bass_kernel_spmd

H = 256
F = 128
B = 32
T = 2048
NCORES = 8
BC = B // NCORES  # 4
GATE = 4 * H  # 1024; gate order follows the reference split: i, f, o, g
CH = 128  # steps per output-staging block

BF16 = mybir.dt.bfloat16
F32 = mybir.dt.float32
AF = mybir.ActivationFunctionType
OP = mybir.AluOpType

_prog_cache = {}


def _build_program(t_steps):
    nc = bacc.Bacc("TRN2", target_bir_lowering=False, debug=False)
    tb = t_steps * BC

    xT = nc.dram_tensor("xT", [F, tb], BF16, kind="ExternalInput")
    w1 = nc.dram_tensor("W1", [4, 128, GATE], BF16, kind="ExternalInput")
    wco = nc.dram_tensor("WcoT", [2, 128, H], BF16, kind="ExternalInput")
    wx = nc.dram_tensor("WxT", [F, GATE], BF16, kind="ExternalInput")
    bias = nc.dram_tensor("bias8", [F, 8], F32, kind="ExternalInput")
    ident = nc.dram_tensor("ident", [128, 128], BF16, kind="ExternalInput")
    out_h = nc.dram_tensor("out_h", [128, t_steps * 8], F32, kind="ExternalOutput")
    out_c = nc.dram_tensor("out_c", [128, t_steps * 8], F32, kind="ExternalOutput")

    n_chunks = tb // 512  # phase-1 moving chunks (512 = 128 steps * 4 b)

    with tile.TileContext(nc) as tc:
        with (
            tc.tile_pool(name="const", bufs=1) as cpool,
            tc.tile_pool(name="xwp", bufs=1) as xwpool,
            tc.tile_pool(name="state", bufs=1) as spool,
            tc.tile_pool(name="xin", bufs=3) as xpool,
            tc.tile_pool(name="ps1", bufs=4, space=bass.MemorySpace.PSUM) as ps1,
            tc.tile_pool(name="gat", bufs=3) as gpool,
            tc.tile_pool(name="stg", bufs=3) as stpool,
            tc.tile_pool(name="ps2", bufs=4, space=bass.MemorySpace.PSUM) as ps2,
        ):
            # ---- constants ----
            w1_sb = cpool.tile([128, 4 * GATE], BF16, tag="w1")
            for kz in range(4):
                nc.sync.dma_start(w1_sb[:, kz * GATE:(kz + 1) * GATE], w1[kz])
            wco_sb = cpool.tile([128, 2 * H], BF16, tag="wco")
            for k in range(2):
                nc.sync.dma_start(wco_sb[:, k * H:(k + 1) * H], wco[k])
            wx_sb = cpool.tile([128, GATE], BF16, tag="wx")
            nc.sync.dma_start(wx_sb[:], wx.ap())
            bias_sb = cpool.tile([128, 8], F32, tag="bias")
            nc.sync.dma_start(bias_sb[:], bias.ap())
            id_sb = cpool.tile([128, 128], BF16, tag="ident")
            nc.sync.dma_start(id_sb[:], ident.ap())

            # ---- phase 1: xw = x @ Wx.T + bias (bf16, SBUF-resident) ----
            xw_sb = xwpool.tile([128, t_steps * 32], BF16, tag="xw")
            xw3 = xw_sb[:].rearrange("p (s g) -> p s g", g=32)
            for n in range(n_chunks):
                xchunk = xpool.tile([128, 512], BF16, tag="xchunk")
                nc.sync.dma_start(xchunk[:], xT.ap()[:, n * 512:(n + 1) * 512])
                for m in range(8):
                    ps = ps1.tile([128, 512], F32, tag="ps1")
                    nc.tensor.matmul(
                        ps[:], wx_sb[:, m * 128:(m + 1) * 128], xchunk[:],
                        start=True, stop=True,
                    )
                    src = ps[:].rearrange("p (s b) -> p s b", b=4)
                    dst = xw3[:, n * 128:(n + 1) * 128, m * 4:(m + 1) * 4]
                    nc.scalar.activation(
                        dst, src, AF.Identity, bias=bias_sb[:, m:m + 1]
                    )

            # ---- phase 2: recurrence ----
            h_bf = spool.tile([128, 8], BF16, tag="h_bf")
            c_bf = spool.tile([128, 8], BF16, tag="c_bf")
            c0 = spool.tile([128, 8], F32, tag="c0")
            nc.gpsimd.memset(h_bf[:], 0.0)
            nc.gpsimd.memset(c_bf[:], 0.0)
            nc.gpsimd.memset(c0[:], 0.0)

            c_prev = c0[:]
            stage_h = stage_c = None
            for s in range(t_steps):
                blk, off = divmod(s, CH)
                if off == 0:
                    stage_h = stpool.tile([128, CH * 8], F32, tag="stage_h")
                    stage_c = stpool.tile([128, CH * 8], F32, tag="stage_c")

                ps = ps2.tile([128, 32], F32, tag="ps2")
                # xw injection (identity matmul) — also clears the bank region
                nc.tensor.matmul(
                    ps[:], id_sb[:], xw_sb[:, s * 32:(s + 1) * 32],
                    start=True, stop=False,
                )
                # peepholes: i,f gate tiles (m=0..3), c halves (kz=2,3)
                for m in range(4):
                    for kc in range(2):
                        nc.tensor.matmul(
                            ps[:, m * 4:(m + 1) * 4],
                            w1_sb[:, (2 + kc) * GATE + m * 128:(2 + kc) * GATE + (m + 1) * 128],
                            c_bf[:, kc * 4:(kc + 1) * 4],
                            start=False, stop=False,
                        )
                # h part: all gate tiles (m=0..7), h halves (kh=0,1)
                for m in range(8):
                    for kh in range(2):
                        nc.tensor.matmul(
                            ps[:, m * 4:(m + 1) * 4],
                            w1_sb[:, kh * GATE + m * 128:kh * GATE + (m + 1) * 128],
                            h_bf[:, kh * 4:(kh + 1) * 4],
                            start=False, stop=False,
                        )
                # i, f sigmoid; g tanh
                if_s = gpool.tile([128, 16], F32, tag="if_s")
                nc.scalar.activation(if_s[:], ps[:, 0:16], AF.Sigmoid)
                g_s = gpool.tile([128, 8], F32, tag="g_s")
                nc.scalar.activation(g_s[:], ps[:, 24:32], AF.Tanh)
                # c_new = f*c + i*g
                t1 = gpool.tile([128, 8], F32, tag="t1")
                nc.vector.tensor_mul(t1[:], if_s[:, 0:8], g_s[:])
                t2 = gpool.tile([128, 8], F32, tag="t2")
                nc.vector.tensor_mul(t2[:], if_s[:, 8:16], c_prev)
                c_slice = stage_c[:, off * 8:(off + 1) * 8]
                nc.vector.tensor_add(c_slice, t1[:], t2[:])
                nc.vector.tensor_copy(c_bf[:], c_slice)
                # o peephole: Wco @ c_new into o region (cols 16:24)
                for m in range(2):
                    for k in range(2):
                        nc.tensor.matmul(
                            ps[:, 16 + m * 4:16 + (m + 1) * 4],
                            wco_sb[:, k * H + m * 128:k * H + (m + 1) * 128],
                            c_bf[:, k * 4:(k + 1) * 4],
                            start=False, stop=(m == 1 and k == 1),
                        )
                o_s = gpool.tile([128, 8], F32, tag="o_s")
                nc.scalar.activation(o_s[:], ps[:, 16:24], AF.Sigmoid)
                tc_s = gpool.tile([128, 8], F32, tag="tc_s")
                nc.scalar.activation(tc_s[:], c_slice, AF.Tanh)
                h_slice = stage_h[:, off * 8:(off + 1) * 8]
                nc.vector.tensor_mul(h_slice, o_s[:], tc_s[:])
                nc.vector.tensor_copy(h_bf[:], h_slice)

                c_prev = c_slice
                if off == CH - 1:
                    base = blk * CH * 8
                    nc.sync.dma_start(out_h.ap()[:, base:base + CH * 8], stage_h[:])
                    nc.sync.dma_start(out_c.ap()[:, base:base + CH * 8], stage_c[:])

    nc.compile()
    return nc


def _pack_weights(Wx, bx, Wh, bh, Wci, bci, Wcf, bcf, Wco, bco):
    bf = ml_dtypes.bfloat16
    WhT = np.ascontiguousarray(Wh.T)  # (256, 1024) [h_dim, gate]
    w1 = np.zeros((4, 128, GATE), np.float32)
    w1[0] = WhT[0:128]
    w1[1] = WhT[128:256]
    ct = np.zeros((256, GATE), np.float32)
    ct[:, 0:H] = Wci.T
    ct[:, H:2 * H] = Wcf.T
    w1[2] = ct[0:128]
    w1[3] = ct[128:256]
    wcoT = np.ascontiguousarray(Wco.T)  # (c_dim, o_dim)
    wco = np.stack([wcoT[0:128], wcoT[128:256]])
    bias = bx + bh + np.concatenate([bci, bcf, bco, np.zeros(H, np.float32)])
    bias8 = np.ascontiguousarray(bias.reshape(8, 128).T, dtype=np.float32)
    return {
        "W1": w1.astype(bf),
        "WcoT": wco.astype(bf),
        "WxT": np.ascontiguousarray(Wx.T).astype(bf),
        "bias8": bias8,
        "ident": np.eye(128, dtype=np.float32).astype(bf),
    }


def kernel(x, Wx, bx, Wh, bh, Wci, bci, Wcf, bcf, Wco, bco):
    x = np.asarray(x, np.float32)
    args = [np.asarray(a, np.float32) for a in (Wx, bx, Wh, bh, Wci, bci, Wcf, bcf, Wco, bco)]
    t_steps = x.shape[1]
    bf = ml_dtypes.bfloat16

    common = _pack_weights(*args)
    in_maps = []
    for c in range(NCORES):
        xc = x[c * BC:(c + 1) * BC]  # (4, T, 128)
        xT = np.ascontiguousarray(xc.transpose(2, 1, 0).reshape(F, t_steps * BC))
        in_maps.append({"xT": xT.astype(bf), **common})

    key = t_steps
    if key not in _prog_cache:
        _prog_cache[key] = _build_program(t_steps)
    nc = _prog_cache[key]

    res = run_bass_kernel_spmd(nc, in_maps, core_ids=list(range(NCORES)))

    hiddens = np.zeros((t_steps + 1, B, H), np.float32)
    memorys = np.zeros((t_steps + 1, B, H), np.float32)
    for c in range(NCORES):
        oh = res.results[c]["out_h"].reshape(128, t_steps, 2, 4)
        oc = res.results[c]["out_c"].reshape(128, t_steps, 2, 4)
        # [p, s, half, b] -> [s, b, half*128+p]
        hiddens[1:, c * BC:(c + 1) * BC] = oh.transpose(1, 3, 2, 0).reshape(t_steps, BC, H)
        memorys[1:, c * BC:(c + 1) * BC] = oc.transpose(1, 3, 2, 0).reshape(t_steps, BC, H)
    return hiddens, memorys
